# revision 6
# baseline (speedup 1.0000x reference)
"""Trainium2 Bass kernel for nn_BFS_Refine (gnn_message_passing, 8 cores).

Exact mathematical simplification of the reference (holds for ANY input
values, proven + verified numerically against the oracle):

  The reference hardcodes col = zeros(N).  With N > 1, layer 0's
  bincount gives counts[0] = N > 1, so cid = 0 and mask = (col == 0) is
  all-True; every node is recolored 0 -> 1.  Layer 1 repeats this with
  color 1 (counts[1] = N > 1), mask again all-True, recolor 1 -> 2.
  So for every input: masks are all-True and colors stay uniform.

  signature: cnt = segment_sum(one_hot(col)[src], dst) puts the
  in-degree of each node in the single column `col`; summing over ALL
  nodes (mask all-True) gives (#edges whose dst survives the scatter)
  = Ev in that column.  sig_mean = Ev/N one-hot at color 1 (layer 0)
  resp. color 2 (layer 1).  Hence, with q = fl32(Ev / N):

      trace = [0, q/4, q/2, 0, ..., 0]          (64 floats)
      gates = sigmoid([alpha_0, alpha_1])       ([2, 1])

  The GIN MLP outputs (h) never reach any returned value - dead code.
  x, W1_*, b1_*, W2_* cannot influence the output.

Device work (the bytes that CAN influence the output): the dst row of
edge_index (6.4 MB) is sharded across the 8 NeuronCores; each core
counts entries with 0 <= dst < N (single unsigned is_lt compare with a
fused free-axis accumulate), reduces to a scalar, and core-side ACT
computes sigmoid(alpha).  Host sums the 8 partial counts (exact: all
intermediate sums are integers < 2^24) and assembles the closed form.
"""

import os
import sys

import numpy as np

for _p in ("/opt/trn_rl_repo", "/root/.axon_site/_ro/trn_rl_repo"):
    if os.path.isdir(_p) and _p not in sys.path:
        sys.path.append(_p)

N_NODES = 100000
D_TRACE = 64
N_CORES = 8
P = 128            # SBUF partitions
CHUNKS = 4
CHUNK_W = 400      # int32 columns per chunk
PER_CORE = CHUNKS * P * CHUNK_W          # 204,800 elements (800 KB)
TOTAL_PAD = N_CORES * PER_CORE           # 1,638,400 >= 1,600,000 edges

_CACHE: dict = {}


def _build_bass():
    """Build + compile the per-core SPMD Bass module (cached per process)."""
    import concourse.bacc as bacc
    import concourse.mybir as mybir
    from concourse.tile import TileContext

    nc = bacc.Bacc(
        "TRN2", target_bir_lowering=False, debug=False, enable_asserts=False
    )

    # uint32 view of dst: negative int32 become >= 2^31, so a single
    # unsigned `< N_NODES` test is exactly (0 <= dst < N) in int32 terms.
    dst_d = nc.dram_tensor(
        "dst", [CHUNKS, P, CHUNK_W], mybir.dt.uint32, kind="ExternalInput"
    ).ap()
    alpha_d = nc.dram_tensor(
        "alphas", [1, 2], mybir.dt.float32, kind="ExternalInput"
    ).ap()
    cnt_d = nc.dram_tensor(
        "counts", [P, 1], mybir.dt.float32, kind="ExternalOutput"
    ).ap()
    gates_d = nc.dram_tensor(
        "gates", [1, 2], mybir.dt.float32, kind="ExternalOutput"
    ).ap()

    with TileContext(nc) as tc:
        with (
            tc.tile_pool(name="acc", bufs=1) as acc_pool,
            tc.tile_pool(name="work", bufs=CHUNKS) as work,
        ):
            part = acc_pool.tile([P, CHUNKS], mybir.dt.float32)
            for c in range(CHUNKS):
                t = work.tile([P, CHUNK_W], mybir.dt.uint32, tag="in")
                nc.sync.dma_start(out=t, in_=dst_d[c])
                v = work.tile([P, CHUNK_W], mybir.dt.float32, tag="mask")
                # v = (dst < N) as 1.0/0.0 (unsigned compare via uint32 view)
                nc.vector.tensor_scalar(
                    out=v,
                    in0=t,
                    scalar1=N_NODES,
                    scalar2=None,
                    op0=mybir.AluOpType.is_lt,
                )
                nc.vector.reduce_sum(
                    out=part[:, c : c + 1], in_=v, axis=mybir.AxisListType.X
                )
            red = acc_pool.tile([P, 1], mybir.dt.float32)
            nc.vector.reduce_sum(out=red, in_=part, axis=mybir.AxisListType.X)
            nc.sync.dma_start(out=cnt_d, in_=red)

            a = acc_pool.tile([1, 2], mybir.dt.float32)
            nc.sync.dma_start(out=a, in_=alpha_d)
            g = acc_pool.tile([1, 2], mybir.dt.float32)
            nc.scalar.activation(
                out=g, in_=a, func=mybir.ActivationFunctionType.Sigmoid
            )
            nc.sync.dma_start(out=gates_d, in_=g)

    nc.compile()
    return nc


def _get_nc():
    nc = _CACHE.get("nc")
    if nc is None:
        nc = _build_bass()
        _CACHE["nc"] = nc
    return nc


def _run_on_device(dst_u32: np.ndarray, alphas: np.ndarray, trace: bool = False):
    """dst_u32: flat uint32 [E]; alphas: float32 [1, 2]. Returns (Ev, gates, results)."""
    from concourse.bass_utils import run_bass_kernel_spmd

    buf = np.full(TOTAL_PAD, N_NODES, np.uint32)  # pad fails the < N test
    buf[: dst_u32.shape[0]] = dst_u32
    shards = buf.reshape(N_CORES, CHUNKS, P, CHUNK_W)

    nc = _get_nc()
    in_maps = [
        {"dst": np.ascontiguousarray(shards[i]), "alphas": alphas}
        for i in range(N_CORES)
    ]
    res = run_bass_kernel_spmd(nc, in_maps, list(range(N_CORES)), trace=trace)
    ev = np.float32(0.0)
    for r in res.results:
        # exact: every partial and partial-sum is an integer < 2^24
        ev = ev + np.float32(np.asarray(r["counts"], np.float32).sum(dtype=np.float64))
    gates = np.asarray(res.results[0]["gates"], np.float32).reshape(2, 1)
    return ev, gates, res


def kernel(
    x,
    edge_index,
    W1_0,
    b1_0,
    W2_0,
    b2_0,
    alpha_0,
    W1_1,
    b1_1,
    W2_1,
    b2_1,
    alpha_1,
):
    edge_index = np.asarray(edge_index)
    dst = np.ascontiguousarray(edge_index[1]).astype(np.int32, copy=False)
    dst_u32 = dst.view(np.uint32).ravel()

    alphas = np.array(
        [[np.float32(np.asarray(alpha_0).reshape(-1)[0]),
          np.float32(np.asarray(alpha_1).reshape(-1)[0])]],
        dtype=np.float32,
    )

    ev, gates, _ = _run_on_device(dst_u32, alphas)

    q = np.float32(ev) / np.float32(N_NODES)
    trace = np.zeros(D_TRACE, np.float32)
    trace[1] = q / np.float32(4.0)
    trace[2] = q / np.float32(2.0)
    return trace, gates
